# revision 4
# baseline (speedup 1.0000x reference)
"""Distributed Trainium2 Bass kernel for GQA causal attention
(S=2048, DIM=4096, NH=32, NKV=8, HD=128), tensor-parallel over heads on 8
NeuronCores.

Per-core program (core c owns q-heads 4c..4c+3 and kv-head c):
  1. QKV projection: qT/kT/vT = W.T-slices @ x.T   (bf16 matmul, f32 psum)
  2. RoPE on q/k via a signed pair-permutation matmul + DVE combine (f32)
  3. PE-transpose vT -> v
  4. Flash-style causal attention in "scores-transposed" layout:
     sT[kv,q] = kT.T q, exp on ACT (no max subtraction; values are small),
     causal mask applied as a 0/1 multiply on diagonal blocks,
     yT[hd,q] += v.T p via float32r matmuls; denominator via ones-matmul.
  5. Normalize, cast bf16, AllGather contributions -> full Y.T [4096, S]
  6. Output projection: core c computes out[:, 512c:512(c+1)] (as outT).

Host side shards/preps inputs (transposes, bf16 casts, cos/sin/mask/perm
tables) and concatenates the 8 output column-slices.
"""

import sys

sys.path.insert(0, "/opt/trn_rl_repo")

import numpy as np
import ml_dtypes

import concourse.bass as bass
import concourse.mybir as mybir
import concourse.tile as tile
from concourse import bacc
from concourse import bass_utils

S, DIM = 2048, 4096
NH, NKV, HD = 32, 8, 128
NCORES = 8
QH = NH // NCORES  # 4 q heads per core
KT = DIM // 128  # 32 contraction tiles
ST = S // 512  # 4 sequence tiles of 512
SCALE = 1.0 / float(np.sqrt(HD))

BF = mybir.dt.bfloat16
F32 = mybir.dt.float32
F32R = mybir.dt.float32r
ALU = mybir.AluOpType
ACTF = mybir.ActivationFunctionType


def r32(ap):
    return ap.bitcast(F32R)


def build_nc():
    nc = bacc.Bacc(
        "TRN2",
        target_bir_lowering=False,
        debug=False,
        enable_asserts=True,
        num_devices=NCORES,
    )

    xt = nc.dram_tensor("xt", [DIM, S], BF, kind="ExternalInput").ap()
    wqkvt = nc.dram_tensor("wqkvt", [DIM, 768], BF, kind="ExternalInput").ap()
    wot = nc.dram_tensor("wot", [DIM, 512], BF, kind="ExternalInput").ap()
    cost = nc.dram_tensor("cost", [128, S], F32, kind="ExternalInput").ap()
    sint = nc.dram_tensor("sint", [128, S], F32, kind="ExternalInput").ap()
    maskt = nc.dram_tensor("maskt", [128, 4, 512], F32, kind="ExternalInput").ap()
    rpermt = nc.dram_tensor("rpermt", [128, 128], F32R, kind="ExternalInput").ap()
    identt = nc.dram_tensor("identt", [128, 128], F32, kind="ExternalInput").ap()
    onest = nc.dram_tensor("onest", [128, 1], F32R, kind="ExternalInput").ap()
    outt = nc.dram_tensor("outt", [512, S], F32, kind="ExternalOutput").ap()

    with tile.TileContext(nc) as tc:
        with (
            tc.tile_pool(name="const", bufs=1) as const,
            tc.tile_pool(name="qkvsb", bufs=1) as qkvsb,
            tc.tile_pool(name="psacc", bufs=6, space="PSUM") as psacc,
            tc.tile_pool(name="psstr", bufs=2, space="PSUM") as psstr,
            tc.tile_pool(name="dram", bufs=1, space="DRAM") as dram,
        ):
            cos_sb = const.tile([128, S], F32)
            sin_sb = const.tile([128, S], F32)
            mask_sb = const.tile([128, 4, 512], F32)
            rperm_sb = const.tile([128, 128], F32R)
            ident_sb = const.tile([128, 128], F32)
            ones_sb = const.tile([128, 1], F32R)
            nc.sync.dma_start(cos_sb, cost)
            nc.sync.dma_start(sin_sb, sint)
            nc.sync.dma_start(mask_sb, maskt)
            nc.sync.dma_start(rperm_sb, rpermt)
            nc.sync.dma_start(ident_sb, identt)
            nc.sync.dma_start(ones_sb, onest)

            # persistent activations (f32 q/k to avoid an extra bf16 round-trip)
            q_sb = qkvsb.tile([128, QH, S], F32)  # rope'd qT, head-major
            k_sb = qkvsb.tile([128, S], F32)  # rope'd kT
            vT_sb = qkvsb.tile([128, S], F32)  # vT (pre-transpose)
            v_sb = qkvsb.tile([128, S], F32)  # v, block-transposed

            # ---------------- phase 1: QKV projections + RoPE ----------------
            with (
                tc.tile_pool(name="wqkv", bufs=1) as wqkv,
                tc.tile_pool(name="xs", bufs=4) as xs,
                tc.tile_pool(name="stg", bufs=4) as stg,
            ):
                w_sb = wqkv.tile([128, KT, 768], BF)
                nc.sync.dma_start(
                    w_sb, wqkvt.rearrange("(kb p) m -> p kb m", p=128)
                )

                xt_r = xt.rearrange("(kb p) s -> p kb s", p=128)

                def rope_tile(src_ps, dst_slice, s0):
                    """src_ps: [128,512] f32 psum (pre-rope). dst_slice: SBUF
                    f32 [128,512] destination. s0: sequence offset."""
                    stage = stg.tile([128, 512], F32, tag="stage")
                    nc.vector.tensor_copy(r32(stage), src_ps)
                    rot = psstr.tile([128, 512], F32, tag="str")
                    nc.tensor.matmul(rot, rperm_sb, r32(stage))
                    t1 = stg.tile([128, 512], F32, tag="ropetmp")
                    nc.vector.tensor_tensor(
                        t1, stage, cos_sb[:, s0 : s0 + 512], ALU.mult
                    )
                    t2 = stg.tile([128, 512], F32, tag="ropetmp2")
                    nc.vector.tensor_tensor(
                        t2, rot, sin_sb[:, s0 : s0 + 512], ALU.mult
                    )
                    nc.vector.tensor_tensor(r32(dst_slice), t1, t2, ALU.add)

                for si in range(ST):
                    s0 = 512 * si
                    ps = [
                        psacc.tile([128, 512], F32, tag="acc", name=f"qkv_ps_{si}_{m}")
                        for m in range(6)
                    ]
                    for k in range(KT):
                        xtile = xs.tile([128, 512], BF, tag="xtile")
                        nc.sync.dma_start(xtile, xt[128 * k : 128 * (k + 1), s0 : s0 + 512])
                        for m in range(6):
                            nc.tensor.matmul(
                                ps[m],
                                w_sb[:, k, 128 * m : 128 * (m + 1)],
                                xtile,
                                start=(k == 0),
                                stop=(k == KT - 1),
                            )
                    for m in range(QH):
                        rope_tile(ps[m], q_sb[:, m, s0 : s0 + 512], s0)
                    rope_tile(ps[QH], k_sb[:, s0 : s0 + 512], s0)
                    nc.vector.tensor_copy(vT_sb[:, s0 : s0 + 512], ps[QH + 1])

            # ---------------- phase 2: transpose v ----------------
            for j in range(S // 128):
                vt_ps = psstr.tile([128, 128], F32, tag="str")
                nc.tensor.transpose(
                    vt_ps, vT_sb[:, 128 * j : 128 * (j + 1)], ident_sb
                )
                nc.vector.tensor_copy(r32(v_sb[:, 128 * j : 128 * (j + 1)]), vt_ps)

            # ---------------- phases 3-4: attention + normalize ----------------
            y_bounce = dram.tile([QH * 128, S], BF)
            y_gather = dram.tile([NCORES * QH * 128, S], BF, addr_space="Shared")

            with (
                tc.tile_pool(name="pp", bufs=3) as pp,
                tc.tile_pool(name="pacc", bufs=2) as paccp,
                tc.tile_pool(name="nrm", bufs=2) as nrm,
                tc.tile_pool(name="wo", bufs=1) as wo,
            ):
                # wo weights DMA'd here so the transfer overlaps attention
                wo_sb = wo.tile([128, KT, 512], BF)
                nc.sync.dma_start(wo_sb, wot.rearrange("(kb p) m -> p kb m", p=128))

                for h in range(QH):
                    for qt in range(ST):
                        s0 = 512 * qt
                        nblocks = 4 * qt + 4
                        yT_ps = psacc.tile(
                            [128, 512], F32, tag="acc", name=f"yT_{h}_{qt}"
                        )
                        p_acc = paccp.tile([128, 512], F32, tag="pacc")
                        for j in range(nblocks):
                            sT = psstr.tile([128, 512], F32, tag="str")
                            nc.tensor.matmul(
                                sT,
                                r32(k_sb[:, 128 * j : 128 * (j + 1)]),
                                r32(q_sb[:, h, s0 : s0 + 512]),
                            )
                            p = pp.tile([128, 512], F32, tag="p")
                            nc.scalar.activation(r32(p), sT, ACTF.Exp, scale=SCALE)
                            if j >= 4 * qt:
                                nc.vector.tensor_tensor(
                                    r32(p), p, mask_sb[:, j - 4 * qt, :], ALU.mult
                                )
                            if j == 0:
                                nc.vector.tensor_copy(r32(p_acc), p)
                            else:
                                nc.vector.tensor_tensor(r32(p_acc), p_acc, p, ALU.add)
                            nc.tensor.matmul(
                                yT_ps,
                                r32(v_sb[:, 128 * j : 128 * (j + 1)]),
                                r32(p),
                                start=(j == 0),
                                stop=(j == nblocks - 1),
                            )
                        denom = psstr.tile([1, 512], F32, tag="str")
                        nc.tensor.matmul(denom, ones_sb, r32(p_acc))
                        recip = nrm.tile([1, 512], F32, tag="recip")
                        nc.vector.reciprocal(recip, denom)
                        recip_bc = nrm.tile([128, 512], F32, tag="recipbc")
                        nc.gpsimd.partition_broadcast(recip_bc, recip)
                        yn = nrm.tile([128, 512], BF, tag="yn")
                        nc.vector.tensor_tensor(yn, yT_ps, recip_bc, ALU.mult)
                        nc.sync.dma_start(
                            y_bounce[128 * h : 128 * (h + 1), s0 : s0 + 512], yn
                        )

                # ---------------- phase 5: allgather ----------------
                nc.gpsimd.collective_compute(
                    "AllGather",
                    ALU.bypass,
                    ins=[y_bounce.opt()],
                    outs=[y_gather.opt()],
                    replica_groups=[list(range(NCORES))],
                )

                # ---------------- phase 6: output projection ----------------
                with (
                    tc.tile_pool(name="ys", bufs=4) as ys,
                    tc.tile_pool(name="osb", bufs=4) as osb,
                ):
                    for si in range(ST):
                        s0 = 512 * si
                        ops = [
                            psacc.tile(
                                [128, 512], F32, tag="acc", name=f"o_ps_{si}_{oc}"
                            )
                            for oc in range(4)
                        ]
                        for k in range(KT):
                            ytile = ys.tile([128, 512], BF, tag="ytile")
                            nc.sync.dma_start(
                                ytile,
                                y_gather[128 * k : 128 * (k + 1), s0 : s0 + 512],
                            )
                            for oc in range(4):
                                nc.tensor.matmul(
                                    ops[oc],
                                    wo_sb[:, k, 128 * oc : 128 * (oc + 1)],
                                    ytile,
                                    start=(k == 0),
                                    stop=(k == KT - 1),
                                )
                        for oc in range(4):
                            otile = osb.tile([128, 512], F32, tag="otile")
                            nc.vector.tensor_copy(otile, ops[oc])
                            nc.sync.dma_start(
                                outt[128 * oc : 128 * (oc + 1), s0 : s0 + 512],
                                otile,
                            )

    nc.compile()
    return nc


def make_in_maps(x, freqs_cis, wq, wk, wv, wo):
    f32 = np.float32
    bf = ml_dtypes.bfloat16
    xt = np.ascontiguousarray(x.T).astype(bf)
    cos = np.ascontiguousarray(np.repeat(freqs_cis[:, :, 0].T, 2, axis=0)).astype(f32)
    sin = np.ascontiguousarray(np.repeat(freqs_cis[:, :, 1].T, 2, axis=0)).astype(f32)
    kvi = np.arange(128, dtype=np.int64)[:, None]
    qi = np.arange(512, dtype=np.int64)[None, :]
    mask = np.stack(
        [(kvi + 128 * d <= qi).astype(f32) for d in range(4)], axis=1
    )  # [128, 4, 512]
    rperm = np.zeros((128, 128), f32)
    for r in range(64):
        rperm[2 * r, 2 * r + 1] = -1.0
        rperm[2 * r + 1, 2 * r] = 1.0
    rpermT = np.ascontiguousarray(rperm.T)
    ident = np.eye(128, dtype=f32)
    ones = np.ones((128, 1), f32)

    in_maps = []
    for c in range(NCORES):
        wqkv = np.concatenate(
            [
                wq[512 * c : 512 * (c + 1), :].T,
                wk[128 * c : 128 * (c + 1), :].T,
                wv[128 * c : 128 * (c + 1), :].T,
            ],
            axis=1,
        ).astype(bf)  # [DIM, 768]
        wot = np.ascontiguousarray(wo[512 * c : 512 * (c + 1), :].T).astype(bf)
        in_maps.append(
            {
                "xt": xt,
                "wqkvt": np.ascontiguousarray(wqkv),
                "wot": wot,
                "cost": cos,
                "sint": sin,
                "maskt": mask,
                "rpermt": rpermT,
                "identt": ident,
                "onest": ones,
            }
        )
    return in_maps


def install_ntff_hook():
    """Inject the missing ``antenv.axon_hooks`` module backed by ctypes calls
    into libaxon_pjrt.so, enabling run_bass_kernel_spmd(trace=True) under
    axon. Also neuter upload_artifacts (no artifact bucket here)."""
    import sys as _sys
    import types
    import ctypes
    import contextlib

    if "antenv.axon_hooks" in _sys.modules:
        return
    so_path = "/opt/axon/libaxon_pjrt.so"
    lib = ctypes.CDLL(so_path)
    lib.axon_start_nrt_profile.argtypes = [
        ctypes.POINTER(ctypes.c_int64),
        ctypes.c_size_t,
    ]
    lib.axon_start_nrt_profile.restype = ctypes.c_int64
    lib.axon_stop_nrt_profile.argtypes = [ctypes.c_char_p]
    lib.axon_stop_nrt_profile.restype = ctypes.c_int64

    @contextlib.contextmanager
    def _hook(output_dir, device_ids):
        import jax

        jax.devices()
        if device_ids:
            ids = (ctypes.c_int64 * len(device_ids))(*device_ids)
            rc = lib.axon_start_nrt_profile(ids, len(device_ids))
        else:
            rc = lib.axon_start_nrt_profile(None, 0)
        if rc != 0:
            raise RuntimeError(f"axon_start_nrt_profile rc={rc}")
        try:
            yield
        finally:
            n = lib.axon_stop_nrt_profile(str(output_dir).encode())
            print(f"ntff profile: {n} file(s) written to {output_dir}")

    mod = types.ModuleType("antenv.axon_hooks")
    mod.get_axon_ntff_profile_hook = lambda: _hook
    mod.set_axon_ntff_profile_hook = lambda h: None
    _sys.modules["antenv.axon_hooks"] = mod
    import antenv

    antenv.axon_hooks = mod
    bass_utils.upload_artifacts = lambda tmpdir: tmpdir


def run(x, freqs_cis, wq, wk, wv, wo, trace=False, trace_kwargs=None):
    if trace:
        install_ntff_hook()
    nc = build_nc()
    in_maps = make_in_maps(x, freqs_cis, wq, wk, wv, wo)
    res = bass_utils.run_bass_kernel_spmd(
        nc,
        in_maps,
        core_ids=list(range(NCORES)),
        trace=trace,
        **(trace_kwargs or {}),
    )
    outs = [r["outt"] for r in res.results]  # each [512, S] = outT slice
    full = np.concatenate([np.asarray(o).T for o in outs], axis=1).astype(np.float32)
    return full, res


def kernel(x, freqs_cis, wq, wk, wv, wo):
    full, _ = run(
        np.asarray(x, np.float32),
        np.asarray(freqs_cis, np.float32),
        np.asarray(wq, np.float32),
        np.asarray(wk, np.float32),
        np.asarray(wv, np.float32),
        np.asarray(wo, np.float32),
    )
    return full


# revision 5
# speedup vs baseline: 1.0630x; 1.0630x over previous
"""Distributed Trainium2 Bass kernel for GQA causal attention
(S=2048, DIM=4096, NH=32, NKV=8, HD=128), tensor-parallel over heads on 8
NeuronCores.

Per-core program (core c owns q-heads 4c..4c+3 and kv-head c):
  1. QKV projection: qT/kT/vT = W.T-slices @ x.T   (bf16 matmul, f32 psum)
  2. RoPE on q/k via a signed pair-permutation matmul + DVE combine,
     output cast to bf16
  3. PE-transpose vT -> v (bf16)
  4. Causal attention in "scores-transposed" layout, all-bf16 matmuls:
     sT[kv,q] = kT.T q; exp on ACT (no max subtraction; scores are small);
     causal mask as a 0/1 bf16 multiply on diagonal blocks;
     denominator accumulated on the PE via a ones-matmul;
     yT[hd,q] += v.T p.
  5. Normalize (reciprocal computed in a [128,4] layout to use all DVE
     lanes), cast bf16, per-head AllGather -> full Y.T [4096, S]
  6. Output projection: core c computes out[:, 512c:512(c+1)] (as outT).

Host side shards/preps inputs (transposes, bf16 casts, cos/sin/mask/perm
tables) and concatenates the 8 output column-slices.
"""

import sys

sys.path.insert(0, "/opt/trn_rl_repo")

import numpy as np
import ml_dtypes

import concourse.bass as bass
import concourse.mybir as mybir
import concourse.tile as tile
from concourse import bacc
from concourse import bass_utils

S, DIM = 2048, 4096
NH, NKV, HD = 32, 8, 128
NCORES = 8
QH = NH // NCORES  # 4 q heads per core
KT = DIM // 128  # 32 contraction tiles
ST = S // 512  # 4 sequence tiles of 512
SCALE = 1.0 / float(np.sqrt(HD))

BF = mybir.dt.bfloat16
F32 = mybir.dt.float32
F32R = mybir.dt.float32r
ALU = mybir.AluOpType
ACTF = mybir.ActivationFunctionType


def r32(ap):
    return ap.bitcast(F32R)


def build_nc():
    nc = bacc.Bacc(
        "TRN2",
        target_bir_lowering=False,
        debug=False,
        enable_asserts=True,
        num_devices=NCORES,
    )

    xt = nc.dram_tensor("xt", [DIM, S], BF, kind="ExternalInput").ap()
    wqkvt = nc.dram_tensor("wqkvt", [DIM, 768], BF, kind="ExternalInput").ap()
    wot = nc.dram_tensor("wot", [DIM, 512], BF, kind="ExternalInput").ap()
    cost = nc.dram_tensor("cost", [128, S], F32, kind="ExternalInput").ap()
    sint = nc.dram_tensor("sint", [128, S], F32, kind="ExternalInput").ap()
    maskt = nc.dram_tensor("maskt", [128, 4, 512], BF, kind="ExternalInput").ap()
    rpermt = nc.dram_tensor("rpermt", [128, 128], F32R, kind="ExternalInput").ap()
    identt = nc.dram_tensor("identt", [128, 128], F32, kind="ExternalInput").ap()
    onest = nc.dram_tensor("onest", [128, 1], BF, kind="ExternalInput").ap()
    outt = nc.dram_tensor("outt", [512, S], F32, kind="ExternalOutput").ap()

    with tile.TileContext(nc) as tc:
        with (
            tc.tile_pool(name="const", bufs=1) as const,
            tc.tile_pool(name="qkvsb", bufs=1) as qkvsb,
            tc.tile_pool(name="psacc", bufs=6, space="PSUM") as psacc,
            tc.tile_pool(name="psstr", bufs=2, space="PSUM") as psstr,
            tc.tile_pool(name="dram", bufs=1, space="DRAM") as dram,
        ):
            cos_sb = const.tile([128, S], F32)
            sin_sb = const.tile([128, S], F32)
            mask_sb = const.tile([128, 4, 512], BF)
            rperm_sb = const.tile([128, 128], F32R)
            ident_sb = const.tile([128, 128], F32)
            ones_sb = const.tile([128, 1], BF)
            nc.sync.dma_start(cos_sb, cost)
            nc.sync.dma_start(sin_sb, sint)
            nc.sync.dma_start(mask_sb, maskt)
            nc.sync.dma_start(rperm_sb, rpermt)
            nc.sync.dma_start(ident_sb, identt)
            nc.sync.dma_start(ones_sb, onest)

            # persistent activations, attention operands in bf16
            q_sb = qkvsb.tile([128, QH, S], BF)  # rope'd qT, head-major
            k_sb = qkvsb.tile([128, S], BF)  # rope'd kT
            v_sb = qkvsb.tile([128, S], BF)  # v, block-transposed

            # ---------------- phase 1: QKV projections + RoPE ----------------
            with (
                tc.tile_pool(name="wqkv", bufs=1) as wqkv,
                tc.tile_pool(name="xs", bufs=4) as xs,
                tc.tile_pool(name="stg", bufs=4) as stg,
            ):
                w_sb = wqkv.tile([128, KT, 768], BF)
                wqkvt_r = wqkvt.rearrange("(kb p) m -> p kb m", p=128)
                for k in range(KT):
                    nc.sync.dma_start(w_sb[:, k, :], wqkvt_r[:, k, :])

                def rope_tile(src_ps, dst_slice, s0):
                    """src_ps: [128,512] f32 psum (pre-rope). dst_slice: SBUF
                    bf16 [128,512] destination. s0: sequence offset."""
                    stage = stg.tile([128, 512], F32, tag="stage")
                    nc.vector.tensor_copy(r32(stage), src_ps)
                    rot = psstr.tile([128, 512], F32, tag="str")
                    nc.tensor.matmul(rot, rperm_sb, r32(stage))
                    t1 = stg.tile([128, 512], F32, tag="ropetmp")
                    nc.vector.tensor_tensor(
                        t1, stage, cos_sb[:, s0 : s0 + 512], ALU.mult
                    )
                    t2 = stg.tile([128, 512], F32, tag="ropetmp2")
                    nc.vector.tensor_tensor(
                        t2, rot, sin_sb[:, s0 : s0 + 512], ALU.mult
                    )
                    nc.vector.tensor_tensor(dst_slice, t1, t2, ALU.add)

                for si in range(ST):
                    s0 = 512 * si
                    ps = [
                        psacc.tile([128, 512], F32, tag="acc", name=f"qkv_ps_{si}_{m}")
                        for m in range(6)
                    ]
                    for k in range(KT):
                        xtile = xs.tile([128, 512], BF, tag="xtile")
                        nc.sync.dma_start(
                            xtile, xt[128 * k : 128 * (k + 1), s0 : s0 + 512]
                        )
                        for m in range(6):
                            nc.tensor.matmul(
                                ps[m],
                                w_sb[:, k, 128 * m : 128 * (m + 1)],
                                xtile,
                                start=(k == 0),
                                stop=(k == KT - 1),
                            )
                    for m in range(QH):
                        rope_tile(ps[m], q_sb[:, m, s0 : s0 + 512], s0)
                    rope_tile(ps[QH], k_sb[:, s0 : s0 + 512], s0)
                    # v: psum -> staging sbuf, then 4 PE transposes -> v_sb bf16
                    vstage = stg.tile([128, 512], F32, tag="vstage")
                    nc.vector.tensor_copy(vstage, ps[QH + 1])
                    for jj in range(4):
                        j = 4 * si + jj
                        vt_ps = psstr.tile([128, 128], F32, tag="str")
                        nc.tensor.transpose(
                            vt_ps, vstage[:, 128 * jj : 128 * (jj + 1)], ident_sb
                        )
                        nc.vector.tensor_copy(
                            v_sb[:, 128 * j : 128 * (j + 1)], vt_ps
                        )

            # ---------------- phases 3-5: attention, normalize, allgather ----
            y_bounce = [
                dram.tile([128, S], BF, tag=f"yb{h}", name=f"ybounce{h}")
                for h in range(QH)
            ]
            y_gather = [
                dram.tile(
                    [NCORES * 128, S],
                    BF,
                    addr_space="Shared",
                    tag=f"yg{h}",
                    name=f"ygather{h}",
                )
                for h in range(QH)
            ]

            with (
                tc.tile_pool(name="pp", bufs=3) as pp,
                tc.tile_pool(name="nrm", bufs=2) as nrm,
                tc.tile_pool(name="wo", bufs=1) as wo,
            ):
                # wo weights DMA'd here so the transfer overlaps attention
                wo_sb = wo.tile([128, KT, 512], BF)
                wot_r = wot.rearrange("(kb p) m -> p kb m", p=128)
                for k in range(KT):
                    nc.sync.dma_start(wo_sb[:, k, :], wot_r[:, k, :])

                for h in range(QH):
                    for qt in range(ST):
                        s0 = 512 * qt
                        nblocks = 4 * qt + 4
                        yT_ps = psacc.tile(
                            [128, 512], F32, tag="acc", name=f"yT_{h}_{qt}"
                        )
                        den_ps = psacc.tile(
                            [1, 512], F32, tag="acc", name=f"den_{h}_{qt}"
                        )
                        for j in range(nblocks):
                            sT = psstr.tile([128, 512], F32, tag="str")
                            nc.tensor.matmul(
                                sT,
                                k_sb[:, 128 * j : 128 * (j + 1)],
                                q_sb[:, h, s0 : s0 + 512],
                            )
                            p = pp.tile([128, 512], BF, tag="p")
                            nc.scalar.activation(p, sT, ACTF.Exp, scale=SCALE)
                            if j >= 4 * qt:
                                nc.vector.tensor_tensor(
                                    p, p, mask_sb[:, j - 4 * qt, :], ALU.mult
                                )
                            nc.tensor.matmul(
                                den_ps,
                                ones_sb,
                                p,
                                start=(j == 0),
                                stop=(j == nblocks - 1),
                            )
                            nc.tensor.matmul(
                                yT_ps,
                                v_sb[:, 128 * j : 128 * (j + 1)],
                                p,
                                start=(j == 0),
                                stop=(j == nblocks - 1),
                            )
                        # reciprocal in [128,4] layout (all DVE lanes), then
                        # broadcast across partitions on gpsimd
                        den_sb = nrm.tile([1, 512], F32, tag="densb")
                        nc.vector.tensor_copy(den_sb, den_ps)
                        den_t = nrm.tile([128, 4], F32, tag="dent")
                        nc.sync.dma_start(den_t, den_sb)
                        rec_t = nrm.tile([128, 4], F32, tag="rect")
                        nc.vector.reciprocal(rec_t, den_t)
                        rec_sb = nrm.tile([1, 512], F32, tag="recsb")
                        nc.sync.dma_start(rec_sb, rec_t)
                        recip_bc = nrm.tile([128, 512], F32, tag="recipbc")
                        nc.gpsimd.partition_broadcast(recip_bc, rec_sb)
                        yn = nrm.tile([128, 512], BF, tag="yn")
                        nc.vector.tensor_tensor(yn, yT_ps, recip_bc, ALU.mult)
                        nc.sync.dma_start(y_bounce[h][:, s0 : s0 + 512], yn)
                    nc.gpsimd.collective_compute(
                        "AllGather",
                        ALU.bypass,
                        ins=[y_bounce[h].opt()],
                        outs=[y_gather[h].opt()],
                        replica_groups=[list(range(NCORES))],
                    )

                # ---------------- phase 6: output projection ----------------
                with (
                    tc.tile_pool(name="ys", bufs=4) as ys,
                    tc.tile_pool(name="osb", bufs=4) as osb,
                ):
                    for si in range(ST):
                        s0 = 512 * si
                        ops = [
                            psacc.tile(
                                [128, 512], F32, tag="acc", name=f"o_ps_{si}_{oc}"
                            )
                            for oc in range(4)
                        ]
                        for k in range(KT):
                            ytile = ys.tile([128, 512], BF, tag="ytile")
                            nc.sync.dma_start(
                                ytile,
                                y_gather[k % 4][
                                    128 * (k // 4) : 128 * (k // 4 + 1),
                                    s0 : s0 + 512,
                                ],
                            )
                            for oc in range(4):
                                nc.tensor.matmul(
                                    ops[oc],
                                    wo_sb[:, k, 128 * oc : 128 * (oc + 1)],
                                    ytile,
                                    start=(k == 0),
                                    stop=(k == KT - 1),
                                )
                        for oc in range(4):
                            otile = osb.tile([128, 512], F32, tag="otile")
                            nc.vector.tensor_copy(otile, ops[oc])
                            nc.sync.dma_start(
                                outt[128 * oc : 128 * (oc + 1), s0 : s0 + 512],
                                otile,
                            )

    nc.compile()
    return nc


def make_in_maps(x, freqs_cis, wq, wk, wv, wo):
    f32 = np.float32
    bf = ml_dtypes.bfloat16
    xt = np.ascontiguousarray(x.T).astype(bf)
    cos = np.ascontiguousarray(np.repeat(freqs_cis[:, :, 0].T, 2, axis=0)).astype(f32)
    sin = np.ascontiguousarray(np.repeat(freqs_cis[:, :, 1].T, 2, axis=0)).astype(f32)
    kvi = np.arange(128, dtype=np.int64)[:, None]
    qi = np.arange(512, dtype=np.int64)[None, :]
    mask = np.stack(
        [(kvi + 128 * d <= qi).astype(f32) for d in range(4)], axis=1
    ).astype(bf)  # [128, 4, 512]
    rperm = np.zeros((128, 128), f32)
    for r in range(64):
        rperm[2 * r, 2 * r + 1] = -1.0
        rperm[2 * r + 1, 2 * r] = 1.0
    rpermT = np.ascontiguousarray(rperm.T)
    ident = np.eye(128, dtype=f32)
    ones = np.ones((128, 1), bf)

    in_maps = []
    for c in range(NCORES):
        wqkv = np.concatenate(
            [
                wq[512 * c : 512 * (c + 1), :].T,
                wk[128 * c : 128 * (c + 1), :].T,
                wv[128 * c : 128 * (c + 1), :].T,
            ],
            axis=1,
        ).astype(bf)  # [DIM, 768]
        wot = np.ascontiguousarray(wo[512 * c : 512 * (c + 1), :].T).astype(bf)
        in_maps.append(
            {
                "xt": xt,
                "wqkvt": np.ascontiguousarray(wqkv),
                "wot": wot,
                "cost": cos,
                "sint": sin,
                "maskt": mask,
                "rpermt": rpermT,
                "identt": ident,
                "onest": ones,
            }
        )
    return in_maps


def install_ntff_hook():
    """Inject the missing ``antenv.axon_hooks`` module backed by ctypes calls
    into libaxon_pjrt.so, enabling run_bass_kernel_spmd(trace=True) under
    axon. Also neuter upload_artifacts (no artifact bucket here)."""
    import sys as _sys
    import types
    import ctypes
    import contextlib

    if "antenv.axon_hooks" in _sys.modules:
        return
    so_path = "/opt/axon/libaxon_pjrt.so"
    lib = ctypes.CDLL(so_path)
    lib.axon_start_nrt_profile.argtypes = [
        ctypes.POINTER(ctypes.c_int64),
        ctypes.c_size_t,
    ]
    lib.axon_start_nrt_profile.restype = ctypes.c_int64
    lib.axon_stop_nrt_profile.argtypes = [ctypes.c_char_p]
    lib.axon_stop_nrt_profile.restype = ctypes.c_int64

    @contextlib.contextmanager
    def _hook(output_dir, device_ids):
        import jax

        jax.devices()
        if device_ids:
            ids = (ctypes.c_int64 * len(device_ids))(*device_ids)
            rc = lib.axon_start_nrt_profile(ids, len(device_ids))
        else:
            rc = lib.axon_start_nrt_profile(None, 0)
        if rc != 0:
            raise RuntimeError(f"axon_start_nrt_profile rc={rc}")
        try:
            yield
        finally:
            n = lib.axon_stop_nrt_profile(str(output_dir).encode())
            print(f"ntff profile: {n} file(s) written to {output_dir}")

    mod = types.ModuleType("antenv.axon_hooks")
    mod.get_axon_ntff_profile_hook = lambda: _hook
    mod.set_axon_ntff_profile_hook = lambda h: None
    _sys.modules["antenv.axon_hooks"] = mod
    import antenv

    antenv.axon_hooks = mod
    bass_utils.upload_artifacts = lambda tmpdir: tmpdir


def run(x, freqs_cis, wq, wk, wv, wo, trace=False, trace_kwargs=None):
    if trace:
        install_ntff_hook()
    nc = build_nc()
    in_maps = make_in_maps(x, freqs_cis, wq, wk, wv, wo)
    res = bass_utils.run_bass_kernel_spmd(
        nc,
        in_maps,
        core_ids=list(range(NCORES)),
        trace=trace,
        **(trace_kwargs or {}),
    )
    outs = [r["outt"] for r in res.results]  # each [512, S] = outT slice
    full = np.concatenate([np.asarray(o).T for o in outs], axis=1).astype(np.float32)
    return full, res


def kernel(x, freqs_cis, wq, wk, wv, wo):
    full, _ = run(
        np.asarray(x, np.float32),
        np.asarray(freqs_cis, np.float32),
        np.asarray(wq, np.float32),
        np.asarray(wk, np.float32),
        np.asarray(wv, np.float32),
        np.asarray(wo, np.float32),
    )
    return full
